# revision 1
# baseline (speedup 1.0000x reference)
"""Causal multi-head self-attention (B=2, L=2048, D=1024, h=16, RoPE) on 8 TRN2
NeuronCores, tensor-parallel over heads (2 heads/core), host-side sum of the
per-core partial W_o outputs."""

import sys

try:
    import concourse  # noqa: F401
except ImportError:
    sys.path.insert(0, "/opt/trn_rl_repo")

import numpy as np

import concourse.bass as bass
import concourse.mybir as mybir
import concourse.tile as tile
from concourse import bacc
from concourse.bass import ts
from concourse.bass_utils import run_bass_kernel_spmd

F32 = mybir.dt.float32
F32R = mybir.dt.float32r
ActF = mybir.ActivationFunctionType
Alu = mybir.AluOpType

B, L, D = 2, 2048, 1024
H, DH = 16, 64           # heads, head dim
T = B * L                # 4096 tokens
NC = 8                   # cores
HPC = H // NC            # 2 heads per core
DPC = HPC * DH           # 128 dims per core
NB = L // 512            # 4 q-blocks of 512 per batch
KB = L // 128            # 16 k-blocks of 128 per batch
THETA = 10000.0

# set by test harness: run with trace and record exec time
TRACE = False
LAST_EXEC_NS = None
LAST_RESULTS = None

_cache = {}


def _round_f32r(a: np.ndarray) -> np.ndarray:
    """Round fp32 to the PE's fp32r format (RNE on the low 12 mantissa bits)."""
    b = np.ascontiguousarray(a, dtype=np.float32).view(np.uint32).astype(np.uint64)
    r = ((b + 0x800) & 0xFFFFF000).astype(np.uint32)
    return r.view(np.float32).reshape(a.shape)


def _build_nc():
    nc = bacc.Bacc("TRN2", target_bir_lowering=False, debug=False)

    xT = nc.dram_tensor("xT", [D, T], F32R, kind="ExternalInput")
    wqT = nc.dram_tensor("wqT", [D, DPC], F32R, kind="ExternalInput")
    wkT = nc.dram_tensor("wkT", [D, DPC], F32R, kind="ExternalInput")
    wvT = nc.dram_tensor("wvT", [D, DPC], F32R, kind="ExternalInput")
    woC = nc.dram_tensor("woC", [DPC, D], F32R, kind="ExternalInput")
    cosP = nc.dram_tensor("cosP", [DPC, L], F32, kind="ExternalInput")
    sinP2 = nc.dram_tensor("sinP2", [DPC, L], F32, kind="ExternalInput")
    masks = nc.dram_tensor("masks", [128, 4, 512], F32, kind="ExternalInput")
    ident = nc.dram_tensor("ident", [128, 128], F32R, kind="ExternalInput")
    out = nc.dram_tensor("out", [T, D], F32, kind="ExternalOutput")

    with tile.TileContext(nc) as tc:
        with (
            tc.tile_pool(name="const", bufs=1) as cpool,
            tc.tile_pool(name="xp", bufs=1) as xpool,
            tc.tile_pool(name="qkv", bufs=2) as qkvpool,
            tc.tile_pool(name="vaugp", bufs=1) as vaugpool,
            tc.tile_pool(name="rope", bufs=2) as ropepool,
            tc.tile_pool(name="pexp", bufs=4) as pexppool,
            tc.tile_pool(name="attn", bufs=2) as attnpool,
            tc.tile_pool(name="small", bufs=1) as smallpool,
            tc.tile_pool(name="pvsbp", bufs=2) as pvsbpool,
            tc.tile_pool(name="outp", bufs=2) as outpool,
            tc.tile_pool(name="ps_proj", bufs=2, space="PSUM") as ps_proj,
            tc.tile_pool(name="ps_st", bufs=2, space="PSUM") as ps_st,
            tc.tile_pool(name="ps_pv", bufs=2, space="PSUM") as ps_pv,
            tc.tile_pool(name="ps_wo", bufs=2, space="PSUM") as ps_wo,
        ):
            # ---- persistent constants ----
            wq_t = cpool.tile([128, 8, DPC], F32R)
            wk_t = cpool.tile([128, 8, DPC], F32R)
            wv_t = cpool.tile([128, 8, DPC], F32R)
            for dst, src in ((wq_t, wqT), (wk_t, wkT), (wv_t, wvT)):
                nc.sync.dma_start(dst[:], src.rearrange("(ko ki) m -> ki ko m", ki=128))
            woC_t = cpool.tile([128, D], F32R)
            nc.sync.dma_start(woC_t[:], woC[:, :])
            cos_t = cpool.tile([128, L], F32)
            nc.sync.dma_start(cos_t[:], cosP[:, :])
            sin_t = cpool.tile([128, L], F32)
            nc.sync.dma_start(sin_t[:], sinP2[:, :])
            mask_t = cpool.tile([128, 4, 512], F32)
            nc.sync.dma_start(mask_t[:], masks[:, :, :])
            ident_t = cpool.tile([128, 128], F32R)
            nc.sync.dma_start(ident_t[:], ident[:, :])

            for b in range(B):
                # ---- load xT for this batch ----
                xT_t = xpool.tile([128, 8, L], F32R, tag="xT")
                xT_r = xT.rearrange("(ko ki) t -> ki ko t", ki=128)
                for nb in range(NB):
                    nc.sync.dma_start(
                        xT_t[:, :, ts(nb, 512)],
                        xT_r[:, :, b * L + nb * 512 : b * L + (nb + 1) * 512],
                    )

                # ---- projections (+ fused RoPE for Q, K) ----
                qtf = qkvpool.tile([128, L], F32R, tag="qtf")
                ktf = qkvpool.tile([128, L], F32R, tag="ktf")
                vt = qkvpool.tile([128, L], F32R, tag="vt")
                for wt, dst, is_v in ((wq_t, qtf, False), (wk_t, ktf, False), (wv_t, vt, True)):
                    for nb in range(NB):
                        ps = ps_proj.tile([128, 512], F32, tag="proj")
                        for k in range(8):
                            nc.tensor.matmul(
                                ps[:],
                                wt[:, k, :],
                                xT_t[:, k, ts(nb, 512)],
                                start=(k == 0),
                                stop=(k == 7),
                            )
                        if is_v:
                            nc.vector.tensor_copy(dst[:, ts(nb, 512)], ps[:])
                        else:
                            w = ropepool.tile([128, 512], F32, tag="w")
                            wsw = ropepool.tile([128, 512], F32, tag="wsw")
                            nc.vector.tensor_mul(dst[:, ts(nb, 512)], ps[:], cos_t[:, ts(nb, 512)])
                            nc.vector.tensor_mul(w[:], ps[:], sin_t[:, ts(nb, 512)])
                            # swap the 32-row halves within each head's 64 rows
                            for blk, src_blk in enumerate((1, 0, 3, 2)):
                                nc.sync.dma_start(
                                    wsw[32 * blk : 32 * blk + 32, :],
                                    w[32 * src_blk : 32 * src_blk + 32, :],
                                )
                            nc.vector.tensor_tensor(
                                dst[:, ts(nb, 512)], dst[:, ts(nb, 512)], wsw[:], Alu.add
                            )

                # ---- V_aug: natural-layout V with a ones column per head ----
                vaug = vaugpool.tile([128, KB, 130], F32R, tag="vaug")
                for kb in range(KB):
                    pst = ps_proj.tile([128, 128], F32R, tag="proj", name=f"tp_{b}_{kb}")
                    nc.tensor.transpose(pst[:], vt[:, ts(kb, 128)], ident_t[:])
                    nc.vector.tensor_copy(vaug[:, kb, 0:64], pst[:, 0:64])
                    nc.vector.tensor_copy(vaug[:, kb, 65:129], pst[:, 64:128])
                    nc.vector.tensor_scalar(
                        vaug[:, kb, 64:65], pst[:, 0:1], 0.0, 1.0, Alu.mult, Alu.add
                    )
                    nc.vector.tensor_scalar(
                        vaug[:, kb, 129:130], pst[:, 0:1], 0.0, 1.0, Alu.mult, Alu.add
                    )

                # ---- causal attention, k-partition layout ----
                attn_t = attnpool.tile([128, L], F32R, tag="attnT")
                for j in range(NB):
                    nkb = 4 * j + 4
                    pv_list = [ps_pv.tile([65, 512], F32, tag="pv", name=f"pv_{b}_{j}_{h}") for h in range(HPC)]
                    for kb in range(nkb):
                        mi = kb - 4 * j
                        pexps = []
                        for h in range(HPC):
                            hp = 64 * h
                            st_ps = ps_st.tile([128, 512], F32, tag="st", name=f"st_{b}_{j}_{kb}_{h}")
                            nc.tensor.matmul(
                                st_ps[:],
                                ktf[hp : hp + 64, ts(kb, 128)],
                                qtf[hp : hp + 64, ts(j, 512)],
                                start=True,
                                stop=True,
                            )
                            pexp = pexppool.tile([128, 512], F32R, tag="pexp", name=f"pexp_{b}_{j}_{kb}_{h}")
                            nc.scalar.activation(pexp[:], st_ps[:], ActF.Exp, scale=0.125)
                            if mi >= 0:
                                nc.vector.tensor_tensor(pexp[:], pexp[:], mask_t[:, mi, :], Alu.mult)
                            pexps.append(pexp)
                        for h in range(HPC):
                            nc.tensor.matmul(
                                pv_list[h][:],
                                vaug[:, kb, 65 * h : 65 * h + 65],
                                pexps[h][:],
                                start=(kb == 0),
                                stop=(kb == nkb - 1),
                            )
                    for h in range(HPC):
                        pv_ps = pv_list[h]
                        # copy the whole pv psum (attn rows + l row) to SBUF
                        # immediately so the PSUM bank frees for the next block
                        pvsb = pvsbpool.tile([65, 512], F32, tag="pvsb", name=f"pvsb_{b}_{j}_{h}")
                        nc.vector.tensor_copy(pvsb[:], pv_ps[:])
                        lrow = smallpool.tile([1, 512], F32, tag="lrow")
                        nc.sync.dma_start(lrow[:], pvsb[64:65, :])
                        llh = smallpool.tile([64, 512], F32, tag="llh")
                        nc.gpsimd.partition_broadcast(llh[:], lrow[:])
                        nc.vector.reciprocal(llh[:], llh[:])
                        if h == 0:
                            nc.vector.tensor_mul(
                                attn_t[0:64, ts(j, 512)], pvsb[0:64, :], llh[:]
                            )
                        else:
                            nrm = smallpool.tile([64, 512], F32R, tag="nrm")
                            nc.vector.tensor_mul(nrm[:], pvsb[0:64, :], llh[:])
                            nc.sync.dma_start(attn_t[64:128, ts(j, 512)], nrm[:])
                    # ---- partial W_o for this q-block (drains output early) ----
                    for qb in range(4 * j, 4 * j + 4):
                        for nh in range(2):
                            wo_ps = ps_wo.tile([128, 512], F32, tag="wo", name=f"wo_{b}_{qb}_{nh}")
                            nc.tensor.matmul(
                                wo_ps[:],
                                attn_t[:, ts(qb, 128)],
                                woC_t[:, ts(nh, 512)],
                                start=True,
                                stop=True,
                            )
                            osb = outpool.tile([128, 512], F32, tag="osb", name=f"osb_{b}_{qb}_{nh}")
                            nc.scalar.copy(osb[:], wo_ps[:])
                            nc.sync.dma_start(
                                out[b * L + qb * 128 : b * L + qb * 128 + 128, ts(nh, 512)],
                                osb[:],
                            )

    nc.compile()
    return nc


def _host_inputs(x, W_q, W_k, W_v, W_o, token_positions):
    """Build per-core input maps (host-side layout preprocessing only)."""
    # interleaved->rotate-half permutation of head dims, folded into W_q / W_k
    perm = np.empty(D, dtype=np.int64)
    for gh in range(H):
        base = gh * DH
        for i in range(DH // 2):
            perm[base + i] = base + 2 * i
            perm[base + DH // 2 + i] = base + 2 * i + 1
    Wq_p = np.asarray(W_q, np.float32)[perm, :]
    Wk_p = np.asarray(W_k, np.float32)[perm, :]
    Wv = np.asarray(W_v, np.float32)
    Wo = np.asarray(W_o, np.float32)

    xT = _round_f32r(np.asarray(x, np.float32).reshape(T, D).T)

    # RoPE tables (angles in f32 to match the reference's f32 computation)
    pos = np.asarray(token_positions).astype(np.float32)
    inv_freq = (THETA ** (-(np.arange(DH // 2, dtype=np.float32) * 2.0) / DH)).astype(
        np.float32
    )
    ang = (pos[:, None] * inv_freq[None, :]).astype(np.float32)  # [L, 32]
    cos = np.cos(ang.astype(np.float64)).astype(np.float32)  # [L, 32]
    sin = np.sin(ang.astype(np.float64)).astype(np.float32)
    cosP = np.empty((DPC, L), np.float32)
    sinP2 = np.empty((DPC, L), np.float32)
    for lh in range(HPC):
        r0 = 64 * lh
        cosP[r0 : r0 + 32, :] = cos.T
        cosP[r0 + 32 : r0 + 64, :] = cos.T
        sinP2[r0 : r0 + 32, :] = sin.T          # x1 rows: +sin
        sinP2[r0 + 32 : r0 + 64, :] = -sin.T    # x2 rows: -sin
    cosP = np.ascontiguousarray(cosP)
    sinP2 = np.ascontiguousarray(sinP2)

    # additive causal masks for the 4 diagonal 128x512 block offsets
    r = np.arange(128)[:, None]
    c = np.arange(512)[None, :]
    masks = np.stack(
        [np.where(r + 128 * mi <= c, 1.0, 0.0).astype(np.float32) for mi in range(4)],
        axis=1,
    )  # [128, 4, 512]
    masks = np.ascontiguousarray(masks)

    ident = _round_f32r(np.eye(128, dtype=np.float32))

    in_maps = []
    for core in range(NC):
        sl = slice(DPC * core, DPC * (core + 1))
        in_maps.append(
            {
                "xT": xT,
                "wqT": _round_f32r(Wq_p[sl, :].T),
                "wkT": _round_f32r(Wk_p[sl, :].T),
                "wvT": _round_f32r(Wv[sl, :].T),
                "woC": _round_f32r(Wo[:, sl].T),
                "cosP": cosP,
                "sinP2": sinP2,
                "masks": masks,
                "ident": ident,
            }
        )
    return in_maps


def kernel(x, W_q, W_k, W_v, W_o, token_positions):
    global LAST_EXEC_NS, LAST_RESULTS
    if "nc" not in _cache:
        _cache["nc"] = _build_nc()
    nc = _cache["nc"]
    in_maps = _host_inputs(x, W_q, W_k, W_v, W_o, token_positions)
    res = run_bass_kernel_spmd(nc, in_maps, list(range(NC)), trace=TRACE)
    LAST_EXEC_NS = res.exec_time_ns
    LAST_RESULTS = res
    total = np.zeros((T, D), dtype=np.float64)
    for core in range(NC):
        total += res.results[core]["out"].astype(np.float64)
    return total.reshape(B, L, D).astype(np.float32)



# revision 3
# speedup vs baseline: 1.5128x; 1.5128x over previous
"""Causal multi-head self-attention (B=2, L=2048, D=1024, h=16, RoPE) on 8 TRN2
NeuronCores, tensor-parallel over heads (2 heads/core), host-side sum of the
per-core partial W_o outputs.

v2: fp16 datapath, software-pipelined attention (PV lags scores by 2 steps),
batch-1 projection + deferred W_o matmuls interleaved as PE filler to keep the
tensor engine continuously busy (max pstate), fast reciprocal, early DMA of
first projection inputs."""

import sys
from collections import deque

try:
    import concourse  # noqa: F401
except ImportError:
    sys.path.insert(0, "/opt/trn_rl_repo")

import numpy as np

import concourse.bass as bass
import concourse.mybir as mybir
import concourse.tile as tile
from concourse import bacc
from concourse.bass import ts
from concourse.bass_utils import run_bass_kernel_spmd

F32 = mybir.dt.float32
F16 = mybir.dt.float16
ActF = mybir.ActivationFunctionType
Alu = mybir.AluOpType

B, L, D = 2, 2048, 1024
H, DH = 16, 64           # heads, head dim
T = B * L                # 4096 tokens
NC = 8                   # cores
HPC = H // NC            # 2 heads per core
DPC = HPC * DH           # 128 dims per core
NB = L // 512            # 4 q-blocks of 512 per batch
KB = L // 128            # 16 k-blocks of 128 per batch
THETA = 10000.0

# set by test harness: run with trace and record exec time
TRACE = False
LAST_EXEC_NS = None
LAST_RESULTS = None

_cache = {}


def _build_nc():
    nc = bacc.Bacc("TRN2", target_bir_lowering=False, debug=False)

    xT = nc.dram_tensor("xT", [D, T], F16, kind="ExternalInput")
    wqT = nc.dram_tensor("wqT", [D, DPC], F16, kind="ExternalInput")
    wkT = nc.dram_tensor("wkT", [D, DPC], F16, kind="ExternalInput")
    wvT = nc.dram_tensor("wvT", [D, DPC], F16, kind="ExternalInput")
    woC = nc.dram_tensor("woC", [DPC, D], F16, kind="ExternalInput")
    cosP = nc.dram_tensor("cosP", [DPC, L], F16, kind="ExternalInput")
    sinP2 = nc.dram_tensor("sinP2", [DPC, L], F16, kind="ExternalInput")
    masks = nc.dram_tensor("masks", [128, 4, 512], F16, kind="ExternalInput")
    ident = nc.dram_tensor("ident", [128, 128], F16, kind="ExternalInput")
    out = nc.dram_tensor("out", [T, D], F16, kind="ExternalOutput")

    xT_r = xT.rearrange("(ko ki) t -> ki ko t", ki=128)

    with tile.TileContext(nc) as tc:
        with (
            tc.tile_pool(name="const", bufs=1) as cpool,
            tc.tile_pool(name="xp", bufs=2) as xpool,
            tc.tile_pool(name="qkv", bufs=2) as qkvpool,
            tc.tile_pool(name="vaugp", bufs=2) as vaugpool,
            tc.tile_pool(name="rope", bufs=3) as ropepool,
            tc.tile_pool(name="pexp", bufs=6) as pexppool,
            tc.tile_pool(name="attn", bufs=2) as attnpool,
            tc.tile_pool(name="small", bufs=2) as smallpool,
            tc.tile_pool(name="outp", bufs=4) as outpool,
            tc.tile_pool(name="ps_big", bufs=4, space="PSUM") as ps_big,
            tc.tile_pool(name="ps_pv", bufs=4, space="PSUM") as ps_pv,
        ):
            # ---- constants (issued in dependency-priority order) ----
            wq_t = cpool.tile([128, 8, DPC], F16)
            nc.sync.dma_start(wq_t[:], wqT.rearrange("(ko ki) m -> ki ko m", ki=128))

            # xT for batch 0: 4 chunks of 512 tokens
            xT_ts = []
            for b in range(B):
                xT_ts.append(xpool.tile([128, 8, L], F16, tag="xT", name=f"xT_{b}"))
            for nb in range(NB):
                nc.sync.dma_start(
                    xT_ts[0][:, :, ts(nb, 512)],
                    xT_r[:, :, nb * 512 : (nb + 1) * 512],
                )

            cos_t = cpool.tile([128, L], F16)
            nc.sync.dma_start(cos_t[:], cosP[:, :])
            sin_t = cpool.tile([128, L], F16)
            nc.sync.dma_start(sin_t[:], sinP2[:, :])

            wk_t = cpool.tile([128, 8, DPC], F16)
            nc.sync.dma_start(wk_t[:], wkT.rearrange("(ko ki) m -> ki ko m", ki=128))
            wv_t = cpool.tile([128, 8, DPC], F16)
            nc.sync.dma_start(wv_t[:], wvT.rearrange("(ko ki) m -> ki ko m", ki=128))

            mask_t = cpool.tile([128, 4, 512], F16)
            nc.sync.dma_start(mask_t[:], masks[:, :, :])
            ident_t = cpool.tile([128, 128], F16)
            nc.sync.dma_start(ident_t[:], ident[:, :])
            woC_t = cpool.tile([128, D], F16)
            nc.sync.dma_start(woC_t[:], woC[:, :])

            # xT for batch 1 (overlaps batch-0 projection compute)
            for nb in range(NB):
                nc.sync.dma_start(
                    xT_ts[1][:, :, ts(nb, 512)],
                    xT_r[:, :, L + nb * 512 : L + (nb + 1) * 512],
                )

            # ---- per-batch SBUF tiles ----
            qtf = [qkvpool.tile([128, L], F16, tag="qtf", name=f"qtf_{b}") for b in range(B)]
            ktf = [qkvpool.tile([128, L], F16, tag="ktf", name=f"ktf_{b}") for b in range(B)]
            vt = [qkvpool.tile([128, L], F16, tag="vt", name=f"vt_{b}") for b in range(B)]
            vaug = [vaugpool.tile([128, KB, 130], F16, tag="vaug", name=f"vaug_{b}") for b in range(B)]
            attn_t = [attnpool.tile([128, L], F16, tag="attnT", name=f"attn_{b}") for b in range(B)]

            # ones columns of vaug (cols 64 and 129 of each 130 group), whole tile at once
            for b in range(B):
                ones_ap = vaug[b].rearrange("p k (g c) -> p k g c", g=2)[:, :, :, 64:65]
                nc.vector.memset(ones_ap, 1.0)

            # ---------------- emission helpers ----------------

            def emit_proj_group(b, widx, wt, dst, nb, is_v):
                """8 accumulating matmuls producing one 512-token block of one
                projection, plus its drain (RoPE for q/k, plain copy for v)."""
                ps = ps_big.tile([128, 512], F32, tag="big", name=f"pj_{b}_{widx}_{nb}")
                for k in range(8):
                    nc.tensor.matmul(
                        ps[:],
                        wt[:, k, :],
                        xT_ts[b][:, k, ts(nb, 512)],
                        start=(k == 0),
                        stop=(k == 7),
                    )
                if is_v:
                    nc.vector.tensor_copy(vt[b][:, ts(nb, 512)], ps[:])
                else:
                    w = ropepool.tile([128, 512], F16, tag="w", name=f"w_{b}_{widx}_{nb}")
                    wsw = ropepool.tile([128, 512], F16, tag="wsw", name=f"wsw_{b}_{widx}_{nb}")
                    nc.vector.tensor_mul(w[:], ps[:], sin_t[:, ts(nb, 512)])
                    # swap the 32-row halves within each head's 64 rows
                    for blk, src_blk in enumerate((1, 0, 3, 2)):
                        nc.sync.dma_start(
                            wsw[32 * blk : 32 * blk + 32, :],
                            w[32 * src_blk : 32 * src_blk + 32, :],
                        )
                    nc.vector.tensor_mul(dst[b][:, ts(nb, 512)], ps[:], cos_t[:, ts(nb, 512)])
                    nc.vector.tensor_tensor(
                        dst[b][:, ts(nb, 512)], dst[b][:, ts(nb, 512)], wsw[:], Alu.add
                    )

            def emit_transpose(b, kb):
                """Transpose one 128-token block of V into vaug (token-major)."""
                pst = ps_big.tile([128, 128], F16, tag="big", name=f"tp_{b}_{kb}")
                nc.tensor.transpose(pst[:], vt[b][:, ts(kb, 128)], ident_t[:])
                # single copy into cols {0..63, 65..128} of the 130-col group
                dst = vaug[b][:, kb, :].rearrange("p (g c) -> p g c", g=2)[:, :, 0:64]
                src = pst[:, :].rearrange("p (g c) -> p g c", g=2)
                nc.vector.tensor_copy(dst, src)

            def emit_wo(b, qb, nh, drain_eng):
                """One W_o matmul (128 q-rows x 512 out-dims) + drain + store."""
                wo_ps = ps_big.tile([128, 512], F32, tag="big", name=f"wo_{b}_{qb}_{nh}")
                nc.tensor.matmul(
                    wo_ps[:],
                    attn_t[b][:, ts(qb, 128)],
                    woC_t[:, ts(nh, 512)],
                    start=True,
                    stop=True,
                )
                osb = outpool.tile([128, 512], F16, tag="osb", name=f"osb_{b}_{qb}_{nh}")
                if drain_eng == 0:
                    nc.scalar.copy(osb[:], wo_ps[:])
                else:
                    nc.vector.tensor_copy(osb[:], wo_ps[:])
                nc.sync.dma_start(
                    out[b * L + qb * 128 : b * L + qb * 128 + 128, ts(nh, 512)],
                    osb[:],
                )

            # filler queue: closures emitting one PE work group each
            fill_q = deque()

            def fill(n):
                for _ in range(min(n, len(fill_q))):
                    fill_q.popleft()()

            def emit_proj_phase(b):
                """Full projection for one batch, emitted as a contiguous block
                (used for batch 0; batch 1 goes through the filler queue)."""
                for widx, (wt, dst, is_v) in enumerate(
                    ((wq_t, qtf, False), (wk_t, ktf, False), (wv_t, vt, True))
                ):
                    for nb in range(NB):
                        emit_proj_group(b, widx, wt, dst, nb, is_v)
                for kb in range(KB):
                    emit_transpose(b, kb)

            def queue_proj_phase(b):
                for widx, (wt, dst, is_v) in enumerate(
                    ((wq_t, qtf, False), (wk_t, ktf, False), (wv_t, vt, True))
                ):
                    for nb in range(NB):
                        fill_q.append(
                            lambda b=b, widx=widx, wt=wt, dst=dst, nb=nb, is_v=is_v:
                            emit_proj_group(b, widx, wt, dst, nb, is_v)
                        )
                        if is_v:
                            # transposes of the 4 token-blocks in this nb chunk
                            for kb in range(4 * nb, 4 * nb + 4):
                                fill_q.append(lambda b=b, kb=kb: emit_transpose(b, kb))

            def queue_wo(b, j):
                for i, qb in enumerate(range(4 * j, 4 * j + 4)):
                    for nh in range(2):
                        fill_q.append(
                            lambda b=b, qb=qb, nh=nh, de=(i + nh) % 2:
                            emit_wo(b, qb, nh, de)
                        )

            def emit_normalize(b, j, pv_list):
                """Drain the two PV psums for q-block j, compute 1/l, write the
                normalized attention rows into attn_t."""
                pvsb0 = smallpool.tile([65, 512], F32, tag="pvsb0", name=f"pvsb0_{b}_{j}")
                pvsb1 = smallpool.tile([65, 512], F32, tag="pvsb1", name=f"pvsb1_{b}_{j}")
                nc.scalar.copy(pvsb0[:], pv_list[0][:])
                nc.scalar.copy(pvsb1[:], pv_list[1][:])
                lrA = smallpool.tile([1, 512], F32, tag="lrA", name=f"lrA_{b}_{j}")
                lrB = smallpool.tile([1, 512], F32, tag="lrB", name=f"lrB_{b}_{j}")
                nc.sync.dma_start(lrA[:], pvsb0[64:65, :])
                nc.sync.dma_start(lrB[:], pvsb1[64:65, :])
                linv0 = smallpool.tile([1, 512], F32, tag="linv0", name=f"linv0_{b}_{j}")
                linv1 = smallpool.tile([1, 512], F32, tag="linv1", name=f"linv1_{b}_{j}")
                nc.vector.reciprocal_approx_fast(linv0[:], lrA[:])
                nc.vector.reciprocal_approx_fast(linv1[:], lrB[:])
                llh0 = smallpool.tile([64, 512], F32, tag="llh0", name=f"llh0_{b}_{j}")
                llh1 = smallpool.tile([64, 512], F32, tag="llh1", name=f"llh1_{b}_{j}")
                nc.gpsimd.partition_broadcast(llh0[:], linv0[:])
                nc.gpsimd.partition_broadcast(llh1[:], linv1[:])
                nc.vector.tensor_mul(attn_t[b][0:64, ts(j, 512)], pvsb0[0:64, :], llh0[:])
                nrm = smallpool.tile([64, 512], F16, tag="nrm", name=f"nrm_{b}_{j}")
                nc.vector.tensor_mul(nrm[:], pvsb1[0:64, :], llh1[:])
                nc.sync.dma_start(attn_t[b][64:128, ts(j, 512)], nrm[:])

            def emit_attn_phase(b):
                """Causal attention for one batch with a 2-step PV pipeline lag.
                Scores: stationary = K block [64, 128], moving = Q block
                [64, 512] -> psum [128 kpos, 512 q]."""
                step = 0
                for j in range(NB):
                    nkb = 4 * j + 4
                    pv_list = [
                        ps_pv.tile([65, 512], F32, tag="pv", name=f"pv_{b}_{j}_{h}")
                        for h in range(HPC)
                    ]
                    pending = []  # (kb, [pexp_h0, pexp_h1])
                    for kb in range(nkb):
                        pexps = []
                        for h in range(HPC):
                            hp = 64 * h
                            st_ps = ps_big.tile(
                                [128, 512], F32, tag="big", name=f"st_{b}_{j}_{kb}_{h}"
                            )
                            nc.tensor.matmul(
                                st_ps[:],
                                ktf[b][hp : hp + 64, ts(kb, 128)],
                                qtf[b][hp : hp + 64, ts(j, 512)],
                                start=True,
                                stop=True,
                            )
                            pexp = pexppool.tile(
                                [128, 512], F16, tag="pexp", name=f"pexp_{b}_{j}_{kb}_{h}"
                            )
                            nc.scalar.activation(pexp[:], st_ps[:], ActF.Exp, scale=0.125)
                            mi = kb - 4 * j
                            if mi >= 0:
                                nc.vector.tensor_tensor(
                                    pexp[:], pexp[:], mask_t[:, mi, :], Alu.mult
                                )
                            pexps.append(pexp)
                        pending.append((kb, pexps))
                        # PV matmuls lag the score matmuls by 2 steps
                        if len(pending) > 2:
                            pkb, ppexps = pending.pop(0)
                            for h in range(HPC):
                                nc.tensor.matmul(
                                    pv_list[h][:],
                                    vaug[b][:, pkb, 65 * h : 65 * h + 65],
                                    ppexps[h][:],
                                    start=(pkb == 0),
                                    stop=(pkb == nkb - 1),
                                )
                        step += 1
                        if step % 3 == 0:
                            fill(1)
                    # drain the last two pipeline steps
                    for pkb, ppexps in pending:
                        for h in range(HPC):
                            nc.tensor.matmul(
                                pv_list[h][:],
                                vaug[b][:, pkb, 65 * h : 65 * h + 65],
                                ppexps[h][:],
                                start=(pkb == 0),
                                stop=(pkb == nkb - 1),
                            )
                    fill(1)
                    emit_normalize(b, j, pv_list)
                    # pad a couple of fillers so W_o(j) is not emitted directly
                    # behind its attn_t dependency
                    fill(2)
                    queue_wo(b, j)

            # ---------------- program ----------------
            emit_proj_phase(0)
            queue_proj_phase(1)
            emit_attn_phase(0)
            emit_attn_phase(1)
            # drain any remaining fillers (last W_o blocks)
            fill(len(fill_q))

    nc.compile()
    return nc


def _host_inputs(x, W_q, W_k, W_v, W_o, token_positions):
    """Build per-core input maps (host-side layout preprocessing only)."""
    # interleaved->rotate-half permutation of head dims, folded into W_q / W_k
    perm = np.empty(D, dtype=np.int64)
    for gh in range(H):
        base = gh * DH
        for i in range(DH // 2):
            perm[base + i] = base + 2 * i
            perm[base + DH // 2 + i] = base + 2 * i + 1
    Wq_p = np.asarray(W_q, np.float32)[perm, :]
    Wk_p = np.asarray(W_k, np.float32)[perm, :]
    Wv = np.asarray(W_v, np.float32)
    Wo = np.asarray(W_o, np.float32)

    xT = np.asarray(x, np.float32).reshape(T, D).T.astype(np.float16)
    xT = np.ascontiguousarray(xT)

    # RoPE tables (angles in f32 to match the reference's f32 computation)
    pos = np.asarray(token_positions).astype(np.float32)
    inv_freq = (THETA ** (-(np.arange(DH // 2, dtype=np.float32) * 2.0) / DH)).astype(
        np.float32
    )
    ang = (pos[:, None] * inv_freq[None, :]).astype(np.float32)  # [L, 32]
    cos = np.cos(ang.astype(np.float64)).astype(np.float32)  # [L, 32]
    sin = np.sin(ang.astype(np.float64)).astype(np.float32)
    cosP = np.empty((DPC, L), np.float32)
    sinP2 = np.empty((DPC, L), np.float32)
    for lh in range(HPC):
        r0 = 64 * lh
        cosP[r0 : r0 + 32, :] = cos.T
        cosP[r0 + 32 : r0 + 64, :] = cos.T
        sinP2[r0 : r0 + 32, :] = sin.T          # x1 rows: +sin
        sinP2[r0 + 32 : r0 + 64, :] = -sin.T    # x2 rows: -sin
    cosP16 = np.ascontiguousarray(cosP.astype(np.float16))
    sinP216 = np.ascontiguousarray(sinP2.astype(np.float16))

    # multiplicative causal masks for the 4 diagonal 128x512 block offsets
    r = np.arange(128)[:, None]
    c = np.arange(512)[None, :]
    masks = np.stack(
        [np.where(r + 128 * mi <= c, 1.0, 0.0).astype(np.float16) for mi in range(4)],
        axis=1,
    )  # [128, 4, 512]
    masks = np.ascontiguousarray(masks)

    ident = np.eye(128, dtype=np.float16)

    in_maps = []
    for core in range(NC):
        sl = slice(DPC * core, DPC * (core + 1))
        in_maps.append(
            {
                "xT": xT,
                "wqT": np.ascontiguousarray(Wq_p[sl, :].T.astype(np.float16)),
                "wkT": np.ascontiguousarray(Wk_p[sl, :].T.astype(np.float16)),
                "wvT": np.ascontiguousarray(Wv[sl, :].T.astype(np.float16)),
                "woC": np.ascontiguousarray(Wo[:, sl].T.astype(np.float16)),
                "cosP": cosP16,
                "sinP2": sinP216,
                "masks": masks,
                "ident": ident,
            }
        )
    return in_maps


def kernel(x, W_q, W_k, W_v, W_o, token_positions):
    global LAST_EXEC_NS, LAST_RESULTS
    if "nc" not in _cache:
        _cache["nc"] = _build_nc()
    nc = _cache["nc"]
    in_maps = _host_inputs(x, W_q, W_k, W_v, W_o, token_positions)
    res = run_bass_kernel_spmd(nc, in_maps, list(range(NC)), trace=TRACE)
    LAST_EXEC_NS = res.exec_time_ns
    LAST_RESULTS = res
    total = np.zeros((T, D), dtype=np.float64)
    for core in range(NC):
        total += res.results[core]["out"].astype(np.float64)
    return total.reshape(B, L, D).astype(np.float32)
